# revision 1
# baseline (speedup 1.0000x reference)
"""ConvSwiGLU Trainium2 kernel: tensor-parallel over d_ff across 8 NeuronCores.

Layout strategy (all chosen so no on-device transposes are needed):
  - Each core owns a 512-channel slice of d_ff (gate/up columns, conv channels,
    down rows). Every core sees all 8192 tokens.
  - Activations live as [channels(partition), tokens(free)]: the gate/up matmul
    is psum[c, t] = sum_d Wg[d, c] * xT[d, t] with Wg as stored ([d, f]) as the
    stationary operand and x pre-transposed on the host.
  - The depthwise conv runs along the free (token) axis as 5 fused
    scale-shift-accumulate ops with per-partition filter taps, spread across
    ACT (tap 0), DVE, and GpSimd so no single engine saturates.
  - Down matmul: psum[m, t] = sum_f Wd[f, m] * hact[f, t] with Wd as stored.
    Partial outputs (yT per core) are summed on the host (the d_ff all-reduce).
  - Tokens run in 16 chunks of 512 with a one-chunk software pipeline: chunk
    i's matmul covers tokens [512i-2, 512i+510); the conv's right halo (4
    columns) is copied from chunk i+1's h tile (zero at sequence edges), so
    matmuls are always full 512-column f32r at full PE rate.
"""

import os
import sys
from contextlib import ExitStack

import ml_dtypes
import numpy as np

for _p in ("/root/.axon_site/_ro/trn_rl_repo", "/opt/trn_rl_repo"):
    if os.path.isdir(_p) and _p not in sys.path:
        sys.path.append(_p)

import concourse.bass as bass
import concourse.tile as tile
from concourse import bacc, mybir
from concourse.bass_utils import run_bass_kernel_spmd

F32 = mybir.dt.float32
F32R = mybir.dt.float32r
BF16 = mybir.dt.bfloat16
AF = mybir.ActivationFunctionType
ALU = mybir.AluOpType

B, L, D = 4, 2048, 1024
F = 4096
NCORES = 8
FS = F // NCORES          # 512 channels per core
KSUB = D // 128           # 8 contraction subtiles for gate/up
GRP = FS // 128           # 4 channel groups per core
MSUB = D // 128           # 8 output row subtiles for down matmul
T = 512                   # token chunk
TH = T + 4                # chunk + conv halo (last 4 cols filled from next chunk)
NCH = (B * L) // T        # 16 chunks
PER_SEQ = L // T          # 4 chunks per sequence
K = 5                     # conv taps

_cache = {}


def _build_program():
    """Build + bacc-compile the per-core SPMD Tile program once."""
    nc = bacc.Bacc("TRN2", target_bir_lowering=False, debug=False,
                   enable_asserts=False, num_devices=NCORES)

    xTc = nc.dram_tensor("xTc", [NCH, 128, KSUB, T], F32R, kind="ExternalInput").ap()
    wg = nc.dram_tensor("wgS", [128, KSUB, FS], F32R, kind="ExternalInput").ap()
    wu = nc.dram_tensor("wuS", [128, KSUB, FS], F32R, kind="ExternalInput").ap()
    wd = nc.dram_tensor("wdS", [128, GRP, D], BF16, kind="ExternalInput").ap()
    bg = nc.dram_tensor("bgS", [128, GRP], F32, kind="ExternalInput").ap()
    bu = nc.dram_tensor("buS", [128, GRP], F32, kind="ExternalInput").ap()
    cgw = nc.dram_tensor("cgwS", [128, GRP, K], F32, kind="ExternalInput").ap()
    cuw = nc.dram_tensor("cuwS", [128, GRP, K], F32, kind="ExternalInput").ap()
    cgb = nc.dram_tensor("cgbS", [128, GRP], F32, kind="ExternalInput").ap()
    cub = nc.dram_tensor("cubS", [128, GRP], F32, kind="ExternalInput").ap()
    edge = nc.dram_tensor("edgeS", [128, B, 2, GRP, 4], BF16, kind="ExternalInput").ap()
    yT = nc.dram_tensor("yT", [D, B * L], F32, kind="ExternalOutput").ap()

    with tile.TileContext(nc) as tc, ExitStack() as ctx:
        consts = ctx.enter_context(tc.tile_pool(name="consts", bufs=1))
        xpool = ctx.enter_context(tc.tile_pool(name="x", bufs=2))
        hpool = ctx.enter_context(tc.tile_pool(name="h", bufs=18))
        accpool = ctx.enter_context(tc.tile_pool(name="acc", bufs=8))
        gactpool = ctx.enter_context(tc.tile_pool(name="gact", bufs=3))
        hactpool = ctx.enter_context(tc.tile_pool(name="hact", bufs=2))
        outpool = ctx.enter_context(tc.tile_pool(name="out", bufs=2))
        ps_main = ctx.enter_context(tc.tile_pool(name="psm", bufs=4, space="PSUM"))
        ps_dn = ctx.enter_context(tc.tile_pool(name="psd", bufs=4, space="PSUM"))

        # resident weights / constants
        wg_sb = consts.tile([128, KSUB, FS], F32R)
        wu_sb = consts.tile([128, KSUB, FS], F32R)
        wd_sb = consts.tile([128, GRP, D], BF16)
        bg_sb = consts.tile([128, GRP], F32)
        bu_sb = consts.tile([128, GRP], F32)
        cgw_sb = consts.tile([128, GRP, K], F32)
        cuw_sb = consts.tile([128, GRP, K], F32)
        cgb_sb = consts.tile([128, GRP], F32)
        cub_sb = consts.tile([128, GRP], F32)
        edge_sb = consts.tile([128, B, 2, GRP, 4], BF16)
        for sb, dr in ((wg_sb, wg), (wu_sb, wu), (wd_sb, wd), (bg_sb, bg),
                       (bu_sb, bu), (cgw_sb, cgw), (cuw_sb, cuw),
                       (cgb_sb, cgb), (cub_sb, cub), (edge_sb, edge)):
            nc.sync.dma_start(sb[:], dr)

        h_tiles = {}  # chunk -> [(g,'g'), (g,'u')] h_sb tiles, layout [128, TH]

        def produce(i):
            """matmul1 for chunk i -> biased h tiles (cols [0:T))."""
            xt = xpool.tile([128, KSUB, T], F32R, tag="xt")
            nc.sync.dma_start(xt[:], xTc[i])
            tiles = []
            for g in range(GRP):
                for w_full, bias_sb in ((wg_sb, bg_sb), (wu_sb, bu_sb)):
                    h_ps = ps_main.tile([128, T], F32, tag="h_main")
                    for ks in range(KSUB):
                        nc.tensor.matmul(h_ps[:], w_full[:, ks, g * 128:(g + 1) * 128],
                                         xt[:, ks, :],
                                         start=(ks == 0), stop=(ks == KSUB - 1))
                    h_sb = hpool.tile([128, TH], BF16, tag="h_sb")
                    nc.scalar.activation(h_sb[:, 0:T], h_ps[:], AF.Identity,
                                         bias=bias_sb[:, g:g + 1])
                    # reference zero-pads h at sequence starts: first 2 halo
                    # cols must be 0, not the bias the Identity-copy wrote
                    if i % PER_SEQ == 0:
                        nc.gpsimd.memset(h_sb[:, 0:2], 0.0)
                    tiles.append(h_sb)
            h_tiles[i] = tiles

        def consume(i):
            """right-halo fill + conv + swiglu + down matmul for chunk i."""
            cur = h_tiles.pop(i)
            nxt = h_tiles.get(i + 1)
            hact = hactpool.tile([128, GRP, T], BF16, tag="hact")
            for g in range(GRP):
                accs = []
                for ci, (side, tapw) in enumerate((("g", cgw_sb), ("u", cuw_sb))):
                    h_sb = cur[2 * g + ci]
                    if i % PER_SEQ == PER_SEQ - 1:
                        # tokens L-2, L-1 are in no chunk's main region; their
                        # h (host-computed) + 2 zero-pad cols come via edgeS
                        nc.vector.tensor_copy(h_sb[:, T:TH],
                                              edge_sb[:, i // PER_SEQ, ci, g, :])
                    else:
                        nc.vector.tensor_copy(h_sb[:, T:TH], nxt[2 * g + ci][:, 0:4])
                    acc = accpool.tile([128, T], BF16, tag="acc")
                    nc.scalar.mul(acc[:], h_sb[:, 0:T], tapw[:, g, 0:1])
                    for j in range(1, K):
                        nc.vector.scalar_tensor_tensor(
                            acc[:], h_sb[:, j:j + T], tapw[:, g, j:j + 1], acc[:],
                            ALU.mult, ALU.add)
                    accs.append(acc)
                gact = gactpool.tile([128, T], BF16, tag="gact")
                nc.scalar.activation(gact[:], accs[0][:], AF.Silu,
                                     bias=cgb_sb[:, g:g + 1])
                nc.vector.scalar_tensor_tensor(
                    hact[:, g, :], accs[1][:], cub_sb[:, g:g + 1], gact[:],
                    ALU.add, ALU.mult)
            out_sb = outpool.tile([128, MSUB, T], F32, tag="out")
            for ms in range(MSUB):
                dn_ps = ps_dn.tile([128, T], F32, tag="dn")
                for g in range(GRP):
                    nc.tensor.matmul(dn_ps[:], wd_sb[:, g, ms * 128:(ms + 1) * 128],
                                     hact[:, g, :],
                                     start=(g == 0), stop=(g == GRP - 1))
                nc.scalar.copy(out_sb[:, ms, :], dn_ps[:])
            nc.sync.dma_start(
                yT.rearrange("(ms p) t -> p ms t", p=128)[:, :, i * T:(i + 1) * T],
                out_sb[:])

        for i in range(NCH):
            produce(i)
            if i >= 1:
                consume(i - 1)
        consume(NCH - 1)

    nc.compile()
    return nc


def _prep_inputs(x, Wg, bgv, Wu, buv, convg_w, convg_b, convu_w, convu_b, Wd):
    """Host-side shard/layout. Returns list of per-core in_maps."""
    x = np.ascontiguousarray(x, np.float32)
    # padded transpose: [B, D, L+4] with zero halo at sequence edges; chunk j
    # of a sequence is cols [T*j, T*j+T) = tokens [T*j-2, T*j+510)
    xp = np.zeros((B, D, L + 4), np.float32)
    xp[:, :, 2:L + 2] = x.transpose(0, 2, 1)
    blocks = np.stack([xp[:, :, T * j:T * j + T] for j in range(PER_SEQ)], axis=1)
    xTc = np.ascontiguousarray(
        blocks.reshape(NCH, KSUB, 128, T).transpose(0, 2, 1, 3))

    def colsplit(w, c):      # [D, F] -> per-core [128, KSUB, FS]
        s = w[:, c * FS:(c + 1) * FS]
        return np.ascontiguousarray(s.reshape(KSUB, 128, FS).transpose(1, 0, 2))

    def vecsplit(v, c):      # [F] -> [128, GRP]
        return np.ascontiguousarray(v[c * FS:(c + 1) * FS].reshape(GRP, 128).T)

    xe = np.asarray(x, np.float32)[:, L - 2:L, :]          # [B, 2, D]
    in_maps = []
    for c in range(NCORES):
        sl = slice(c * FS, (c + 1) * FS)
        he_g = xe @ np.asarray(Wg, np.float32)[:, sl] + np.asarray(bgv, np.float32)[sl]
        he_u = xe @ np.asarray(Wu, np.float32)[:, sl] + np.asarray(buv, np.float32)[sl]
        edgeS = np.zeros((128, B, 2, GRP, 4), np.float32)
        for s in range(B):
            for ci, he in enumerate((he_g, he_u)):
                # [2, FS] -> [128, GRP, 2] at halo cols 0,1 (tokens L-2, L-1)
                v = he[s].reshape(2, GRP, 128).transpose(2, 1, 0)
                edgeS[:, s, ci, :, 0:2] = v
        edgeS = edgeS.astype(ml_dtypes.bfloat16)
        wdS = Wd[c * FS:(c + 1) * FS, :]
        in_maps.append({
            "edgeS": edgeS,
            "xTc": xTc,
            "wgS": colsplit(np.asarray(Wg, np.float32), c),
            "wuS": colsplit(np.asarray(Wu, np.float32), c),
            "wdS": np.ascontiguousarray(
                np.asarray(wdS, np.float32).reshape(GRP, 128, D).transpose(1, 0, 2)
                .astype(ml_dtypes.bfloat16)),
            "bgS": vecsplit(np.asarray(bgv, np.float32), c),
            "buS": vecsplit(np.asarray(buv, np.float32), c),
            "cgwS": np.ascontiguousarray(
                np.asarray(convg_w, np.float32)[c * FS:(c + 1) * FS, 0, :]
                .reshape(GRP, 128, K).transpose(1, 0, 2)),
            "cuwS": np.ascontiguousarray(
                np.asarray(convu_w, np.float32)[c * FS:(c + 1) * FS, 0, :]
                .reshape(GRP, 128, K).transpose(1, 0, 2)),
            "cgbS": vecsplit(np.asarray(convg_b, np.float32), c),
            "cubS": vecsplit(np.asarray(convu_b, np.float32), c),
        })
    return in_maps


def run_on_cores(in_maps, **kwargs):
    if "nc" not in _cache:
        _cache["nc"] = _build_program()
    return run_bass_kernel_spmd(_cache["nc"], in_maps,
                                core_ids=list(range(NCORES)), **kwargs)


def kernel(x, Wg, bg, Wu, bu, convg_w, convg_b, convu_w, convu_b, Wd, bd):
    in_maps = _prep_inputs(x, Wg, bg, Wu, bu, convg_w, convg_b,
                           convu_w, convu_b, Wd)
    res = run_on_cores(in_maps)
    acc = np.zeros((D, B * L), np.float64)
    for r in res.results:
        acc += r["yT"]
    acc += np.asarray(bd, np.float64)[:, None]
    return np.ascontiguousarray(acc.T.reshape(B, L, D)).astype(np.float32)

